# revision 59
# baseline (speedup 1.0000x reference)
"""BiLSTM-CRF loss kernel for Trainium2, 8 NeuronCores, single launch.

Direction-split data parallelism: cores 0-3 run the FORWARD LSTM for batch
quadrant c (16 batches each); cores 4-7 run the BACKWARD LSTM for quadrant
c-4 (the host hands them time-reversed tokens, so the SPMD program is
identical). Per core: embedding gather, fp8 DoubleRow input projection (bias
folded into the matmul), 512-step LSTM recurrence (fp8 DoubleRow recurrent
matmuls, pre-activations injected into PSUM via identity matmul, fused f+i
sigmoid, hidden-half-pipelined tail), and its direction's half of the
emissions in transposed orientation. Partial emissions are scattered via
host-supplied index maps (which also undo the backward cores' time reversal)
into a canonical [B*S, 64] buffer, summed across cores with an 8-core
AllReduce, then each core gathers the 8 batches it owns and computes the
gold score + exp-space CRF forward (2 interleaved alpha chains, periodic
renormalization). Output: per-core [1, 8] NLL; the host returns the mean.
"""
import os
import sys
from contextlib import ExitStack

import numpy as np

for _p in ("/opt/trn_rl_repo", "/root/.axon_site/_ro/trn_rl_repo"):
    if os.path.isdir(_p) and _p not in sys.path:
        sys.path.append(_p)

import concourse.bass as bass
import concourse.tile as tile
from concourse import bacc, mybir
from concourse.bass_utils import run_bass_kernel_spmd
from concourse.masks import make_identity

F32 = mybir.dt.float32
BF16 = mybir.dt.bfloat16
FP8 = mybir.dt.float8e4
I32 = mybir.dt.int32
AF = mybir.ActivationFunctionType
ALU = mybir.AluOpType
DR = mybir.MatmulPerfMode.DoubleRow

WSCALE = 16.0            # host scales w_ih/w_hh/bias by this; ACT divides out

V, E, H, HID, TT = 50257, 512, 512, 1024, 64
B, S = 64, 512
NCORES = 8
VS = 4096                # compact-table shard rows per core
VPAD = NCORES * VS       # compact table capacity (>= B*S, the max distinct
                         # tokens per call; host remaps token ids into it)

# weight-pack layout: fp8 rows of 512. Both directions in one buffer,
# AllGathered 8-wide; each core reads its direction's half via predicated DMA.
WG_WIH = 0               # 2048 rows (wihT viewed as [2048, 512])
WG_WHH = 2048            # 2048 rows
WG_WOUT = 4096           # 64 rows (woutT viewed as [64, 512])
WG_DIR = 4352            # rows per direction (padded to 4 x 1088)
WG_TOT = 2 * WG_DIR      # fwd pack | bwd pack
WG_PER = WG_TOT // 8     # rows uploaded per core

# int32 blob layout (row offsets). No CRF gather map: each core's CRF block
# is contiguous in the canonical em layout and read via a dynamic-offset DMA
# (row offset = partition_id * 4096), in (b, t) order.
BI_TOK = 0               # 8192 tok_idx
BI_SCAT = 8192           # 8192 scatter map
BI_TOT = 16384
# f32 blob layout
BF_BIAS = 0              # 2048
BF_BOUT = 2048           # 64
BF_TRANS = 2112          # 4096 transitions^T row-major
BF_TNEXT = 6208          # 4096
BF_TPREV = 10304         # 4096
BF_TOT = 14400
BC = 16                  # batch per core in launch 1 (one quadrant)
BCRF = 8                 # batch per core in launch 2
G4 = 4 * H               # gate width (i,f,o,g order after host permutation)
NEG = -10000.0
C_TRANS = 4.0
RENORM = 12

# gate slices after host permutation to (f, i, o, g) — f,i,o contiguous so a
# single sigmoid ACT covers all three
SL_F = slice(0, 512)
SL_I = slice(512, 1024)
SL_O = slice(1024, 1536)
SL_G = slice(1536, 2048)


def build_lstm_program(nsteps=S):
    """Launch-1 program: one LSTM direction, 16 batches, partial emissions."""
    Sq = nsteps
    TOKq = Sq * BC
    NTILE = TOKq // 128
    nc = bacc.Bacc("TRN2", target_bir_lowering=False, debug=False,
                   num_devices=NCORES)

    din = lambda name, shp, dt=F32: nc.dram_tensor(name, shp, dt, kind="ExternalInput").ap()
    blob_i32 = din("blob_i32", [BI_TOT, 1], I32)   # tok_idx | scat_map | gath_map
    emb_shard = din("emb_shard", [VS, E], FP8)     # this core's vocab rows, fp8
    wg_shard = din("wg_shard", [WG_PER, 512], FP8)  # direction-group weight shard
    blob_f32 = din("blob_f32", [BF_TOT, 1])  # bias|bout|transT|tags_next|tags_prev
    out_dram = nc.dram_tensor("out", [1, BCRF], F32, kind="ExternalOutput").ap()

    pre = nc.dram_tensor("pre", [TOKq, G4], FP8).ap()
    em_loc = nc.dram_tensor("em_loc", [B * S, TT], BF16).ap()
    em_red = nc.dram_tensor("em_red", [B * S, TT], BF16, addr_space="Shared").ap()
    ag_in = nc.dram_tensor("ag_in", [VS, E], BF16).ap()
    emb_full = nc.dram_tensor("emb_full", [VPAD, E], BF16,
                              addr_space="Shared").ap()
    ag2_in = nc.dram_tensor("ag2_in", [WG_PER, 512], FP8).ap()
    wg_full = nc.dram_tensor("wg_full", [WG_TOT, 512], FP8,
                             addr_space="Shared").ap()

    with tile.TileContext(nc) as tc, ExitStack() as es:
        # ======== Phase 0: assemble embedding table + weights ====
        # each core uploads V/8 vocab rows (fp8) and a quarter of its
        # direction's weights; AllGathers rebuild both in local DRAM so the
        # per-call input-transfer cost collapses. The emb table is widened
        # to bf16 during staging so the phase-A transpose path is contiguous.
        with tc.tile_pool(name="stage", bufs=4) as stp:
            for i in range(VS // 128):
                sg = stp.tile([128, E], FP8, tag="stg")
                nc.sync.dma_start(sg[:], emb_shard[i * 128:(i + 1) * 128, :])
                sgb = stp.tile([128, E], BF16, tag="stgb")
                nc.vector.tensor_copy(sgb[:], sg[:])
                nc.sync.dma_start(ag_in[i * 128:(i + 1) * 128, :], sgb[:])
            for i in range(WG_PER // 128):
                wsg = stp.tile([128, 512], FP8, tag="wsg")
                nc.sync.dma_start(wsg[:], wg_shard[i * 128:(i + 1) * 128, :])
                nc.sync.dma_start(ag2_in[i * 128:(i + 1) * 128, :], wsg[:])
            rem = WG_PER % 128
            if rem:
                wsr = stp.tile([128, 512], FP8, tag="wsg")
                nc.sync.dma_start(wsr[:rem, :], wg_shard[WG_PER - rem:, :])
                nc.sync.dma_start(ag2_in[WG_PER - rem:, :], wsr[:rem, :])
        nc.gpsimd.collective_compute(
            "AllGather", mybir.AluOpType.bypass,
            replica_groups=[[0, 1, 2, 3, 4, 5, 6, 7]],
            ins=[ag_in[:]], outs=[emb_full[:]])
        nc.gpsimd.collective_compute(
            "AllGather", mybir.AluOpType.bypass,
            replica_groups=[[0, 1, 2, 3, 4, 5, 6, 7]],
            ins=[ag2_in[:]], outs=[wg_full[:]])
        # fwd cores (rank < 4) read the fwd half of the weight pack, bwd
        # cores the bwd half — same program, predicated DMAs
        pid = nc.sync.partition_id()
        cond_fwd = pid < 4
        cond_bwd = pid >= 4

        def load_w(dst, region, k, rows, pattern, **axes):
            for base, cnd in ((region, cond_fwd), (WG_DIR + region, cond_bwd)):
                nc.sync.dma_start(
                    dst, wg_full[base + rows * k:base + rows * (k + 1), :]
                    .rearrange(pattern, **axes), cond=cnd)

        cpool = es.enter_context(tc.tile_pool(name="const", bufs=1))
        identf = cpool.tile([128, 128], F32)
        make_identity(nc, identf[:])
        identb = cpool.tile([128, 128], BF16)
        nc.vector.tensor_copy(identb[:], identf[:])
        ones_row = cpool.tile([1, 512], BF16)
        nc.vector.memset(ones_row[:], 1.0)
        identp = cpool.tile([BC, BC], FP8)
        nc.vector.tensor_copy(identp[:], identf[:BC, :BC])

        # persistent across phases: h history (transposed), one tile per
        # DoubleRow K-chunk pair so step t+1's pair-0 matmuls only wait on
        # the pair-0 copies of step t
        pph = es.enter_context(tc.tile_pool(name="pph", bufs=1))
        hpair = [pph.tile([128, 2 * TOKq], FP8, tag=f"hh{j}", name=f"hh{j}")
                 for j in range(2)]

        # ======== Phase A+B scope: gather, input projection ====
        with tc.tile_pool(name="ppx", bufs=1) as ppx:
            xT = ppx.tile([128, 4 * TOKq], FP8, tag="xT", name="xT")
            xT3 = xT[:].rearrange("p (k n) -> p k n", k=4)

            # gather + projection pools open together (PSUM 4 + 4 banks) so
            # the scheduler can overlap phase A tiles with phase B matmuls
            with tc.tile_pool(name="gather", bufs=4) as gp, \
                 tc.tile_pool(name="pgather", bufs=4, space="PSUM") as pg, \
                 tc.tile_pool(name="iproj", bufs=3) as ip, \
                 tc.tile_pool(name="ipw", bufs=1) as ipw, \
                 tc.tile_pool(name="pproj", bufs=2, space="PSUM") as ppj:
                for i in range(NTILE):
                    idx_t = gp.tile([128, 1], I32, tag="idx")
                    nc.sync.dma_start(
                        idx_t[:], blob_i32[BI_TOK + i * 128:BI_TOK + (i + 1) * 128, :])
                    xg = gp.tile([128, E], BF16, tag="xg")
                    nc.gpsimd.indirect_dma_start(
                        out=xg[:], out_offset=None, in_=emb_full[:],
                        in_offset=bass.IndirectOffsetOnAxis(ap=idx_t[:, :1], axis=0))
                    for k in range(4):
                        tp = pg.tile([128, 128], BF16, tag="tp")
                        nc.tensor.transpose(out=tp[:],
                                            in_=xg[:, k * 128:(k + 1) * 128],
                                            identity=identb[:])
                        nc.vector.tensor_copy(
                            xT[:, k * TOKq + i * 128:k * TOKq + (i + 1) * 128],
                            tp[:])

                # weights for input projection (fp8, DMA'd straight into the
                # DoubleRow-paired layout from the AllGathered pack)
                wihp = [ipw.tile([128, 2 * G4], FP8, tag=f"wihp{j}",
                                 name=f"wihp{j}") for j in range(2)]
                for k in range(4):
                    load_w(wihp[k // 2][:, (k % 2) * G4:(k % 2 + 1) * G4],
                           WG_WIH, k, 512, "(p f) c -> p (f c)", p=128)
                bias_f = ip.tile([1, G4], F32, tag="bias_f")
                nc.sync.dma_start(
                    bias_f[:],
                    blob_f32[BF_BIAS:BF_BIAS + G4, :].rearrange("n 1 -> 1 n"))
                bias_sb = ipw.tile([1, G4], BF16, tag="bias_sb", name="bias_sb")
                nc.vector.tensor_copy(bias_sb[:], bias_f[:])
                for i in range(NTILE):
                    for chp in range(2):
                        # two 512-wide gate chunks share one 2-bank PSUM tile
                        # so the evacuation copy + DMA count halves
                        acc = ppj.tile([128, 1024], F32, tag="acc")
                        for h2 in range(2):
                            ch = 2 * chp + h2
                            csl = slice(ch * 512, (ch + 1) * 512)
                            asl = slice(h2 * 512, (h2 + 1) * 512)
                            nc.tensor.matmul(
                                acc[:, asl], ones_row[:1, :128],
                                bias_sb[:1, csl], start=True, stop=False)
                            for j in range(2):
                                nc.tensor.matmul(
                                    acc[:, asl],
                                    xT3[:, 2 * j:2 * j + 2, i * 128:(i + 1) * 128],
                                    wihp[j][:].rearrange("p (two g) -> p two g",
                                                         two=2)[:, :, csl],
                                    start=False, stop=(j == 1), perf_mode=DR)
                        pre_t = ip.tile([128, 1024], FP8, tag="pre_t")
                        nc.vector.tensor_copy(pre_t[:], acc[:])
                        nc.sync.dma_start(
                            pre[i * 128:(i + 1) * 128,
                                chp * 1024:(chp + 1) * 1024],
                            pre_t[:])

        # ======== Phase C: LSTM recurrence, single direction ====
        # recurrent weights in fp8 (scaled x16 by host), paired for DoubleRow:
        # wpair[j] holds K-chunks 2j (slot 0) and 2j+1 (slot 1)
        with tc.tile_pool(name="recw", bufs=1) as rw:
            wpair = [rw.tile([128, 2 * G4], FP8, tag=f"whhp{j}", name=f"whhp{j}")
                     for j in range(2)]
            for k in range(4):
                load_w(wpair[k // 2][:, (k % 2) * G4:(k % 2 + 1) * G4],
                       WG_WHH, k, 512, "(p f) c -> p (f c)", p=128)

            with tc.tile_pool(name="rec", bufs=3) as rp, \
                 tc.tile_pool(name="recst", bufs=1) as rs, \
                 tc.tile_pool(name="prebuf", bufs=6) as pb, \
                 tc.tile_pool(name="emp", bufs=4) as emp, \
                 tc.tile_pool(name="empw", bufs=1) as empw, \
                 tc.tile_pool(name="pgate", bufs=1, space="PSUM") as pgate, \
                 tc.tile_pool(name="ptr", bufs=2, space="PSUM") as ptr, \
                 tc.tile_pool(name="pem", bufs=2, space="PSUM") as pem:
                c_sb = rs.tile([BC, H], BF16, tag="c", name="c_state")
                nc.vector.memset(c_sb[:], 0.0)

                # emission weights/bias + em zero-fill up front: each
                # 128-token emission tile is produced as soon as its 8 LSTM
                # steps complete, so the slow gpsimd scatter overlaps the
                # recurrence instead of serializing after it
                woutp = [empw.tile([128, 2 * TT], FP8, tag=f"woutp{j}",
                                    name=f"woutp{j}") for j in range(2)]
                for k in range(4):
                    load_w(woutp[k // 2][:, (k % 2) * TT:(k % 2 + 1) * TT],
                           WG_WOUT, k, 16, "a (b c) -> (a b) c", b=8)
                bout_f = emp.tile([1, TT], F32, tag="bout_f")
                nc.sync.dma_start(
                    bout_f[:],
                    blob_f32[BF_BOUT:BF_BOUT + TT, :].rearrange("n 1 -> 1 n"))
                bout_sb = empw.tile([1, TT], BF16, tag="bout_sb", name="bout_sb")
                nc.vector.tensor_copy(bout_sb[:], bout_f[:])
                zt = empw.tile([128, 512], BF16, tag="zt", name="zt")
                nc.vector.memset(zt[:], 0.0)
                ZR = (B * S * TT) // (128 * 512)
                em_flat = em_loc.rearrange("r c -> (r c)").rearrange(
                    "(a p q) -> a p q", p=128, q=512)
                for z in range(ZR):
                    nc.sync.dma_start(em_flat[z], zt[:])

                def emit_tile(i):
                    acc = pem.tile([128, TT], F32, tag="emaccT",
                                   name=f"emaccT{i}")
                    nc.tensor.matmul(acc[:], ones_row[:1, :128], bout_sb[:1, :],
                                     start=True, stop=False)
                    for j in range(2):
                        nc.tensor.matmul(
                            acc[:],
                            hpair[j][:].rearrange(
                                "p (k n) -> p k n",
                                k=2)[:, :, i * 128:(i + 1) * 128],
                            woutp[j][:].rearrange("p (two g) -> p two g", two=2),
                            start=False, stop=(j == 1), perf_mode=DR)
                    em_t = emp.tile([128, TT], BF16, tag="em_t")
                    nc.vector.tensor_copy(em_t[:], acc[:])
                    sidx = emp.tile([128, 1], I32, tag="sidx")
                    nc.sync.dma_start(
                        sidx[:],
                        blob_i32[BI_SCAT + i * 128:BI_SCAT + (i + 1) * 128, :])
                    nc.gpsimd.indirect_dma_start(
                        out=em_loc[:], out_offset=bass.IndirectOffsetOnAxis(
                            ap=sidx[:, :1], axis=0),
                        in_=em_t[:], in_offset=None)

                def gate_mms(Gc, csl, t, first_write):
                    nc.tensor.matmul(
                        Gc, identp[:], pre_row[:, csl],
                        start=True, stop=(t == 0))
                    if t > 0:
                        for j in range(2):
                            lhs = hpair[j][:].rearrange(
                                "p (k n) -> p k n", k=2)[:, :,
                                                        (t - 1) * BC:t * BC]
                            rhs = wpair[j][:].rearrange(
                                "p (two g) -> p two g", two=2)[:, :, csl]
                            nc.tensor.matmul(
                                Gc, lhs, rhs,
                                start=False, stop=(j == 1), perf_mode=DR)

                for t in range(Sq):
                    pre_row = pb.tile([BC, G4], FP8, tag="prerow")
                    nc.sync.dma_start(pre_row[:], pre[t * BC:(t + 1) * BC, :])
                    # layout is (f, i, o, g); g gets its own PSUM tile (tanh
                    # leads the ACT block), f+i+o share one tile so a single
                    # sigmoid instruction covers all three
                    Gg = pgate.tile([BC, 512], F32, tag="gg", name=f"Gg{t}")
                    gate_mms(Gg[:], SL_G, t, True)
                    g_g = rp.tile([BC, 512], BF16, tag="gag", name=f"gag{t}")
                    nc.scalar.activation(g_g[:], Gg[:], AF.Tanh,
                                         scale=1.0 / WSCALE)
                    Gfio = pgate.tile([BC, 1536], F32, tag="gfio", name=f"Gfio{t}")
                    gate_mms(Gfio[:, 0:512], SL_F, t, True)
                    gate_mms(Gfio[:, 512:1024], SL_I, t, False)
                    gate_mms(Gfio[:, 1024:1536], SL_O, t, False)
                    g_fio = rp.tile([BC, 1536], BF16, tag="gafio", name=f"gafio{t}")
                    nc.scalar.activation(g_fio[:], Gfio[:], AF.Sigmoid,
                                         scale=1.0 / WSCALE)
                    g_f, g_i = g_fio[:, 0:512], g_fio[:, 512:1024]
                    g_o = g_fio[:, 1024:1536]
                    tmp = rp.tile([BC, H], BF16, tag="tmp", name=f"tmp{t}")
                    nc.vector.tensor_mul(tmp[:], g_i, g_g[:])
                    nc.vector.tensor_mul(c_sb[:], g_f, c_sb[:])
                    nc.vector.tensor_add(c_sb[:], c_sb[:], tmp[:])
                    # tanh(c), h, transpose, hhist copy all split by hidden
                    # halves so the next step's first DoubleRow pair (K-chunks
                    # 0,1) can start as soon as half 0 lands in hhist
                    for hf in range(2):
                        hsl = slice(hf * 256, (hf + 1) * 256)
                        tanc = rp.tile([BC, 256], BF16, tag=f"tanc{hf}",
                                       name=f"tanc{t}_{hf}")
                        nc.scalar.activation(tanc[:], c_sb[:, hsl], AF.Tanh)
                        h_sb = rp.tile([BC, 256], BF16, tag=f"hsb{hf}",
                                       name=f"hsb{t}_{hf}")
                        nc.vector.tensor_mul(h_sb[:], g_o[:, hsl], tanc[:])
                        for kk in range(2):
                            tp = ptr.tile([128, BC], BF16, tag="htp",
                                          name=f"htp{t}_{2 * hf + kk}")
                            nc.tensor.transpose(
                                out=tp[:], in_=h_sb[:, kk * 128:(kk + 1) * 128],
                                identity=identb[:BC, :BC])
                            nc.vector.tensor_copy(
                                hpair[hf][:, kk * TOKq + t * BC:
                                           kk * TOKq + (t + 1) * BC],
                                tp[:])
                    if t % 8 == 7:
                        emit_tile(t // 8)

        nc.gpsimd.collective_compute(
            "AllReduce", mybir.AluOpType.add,
            replica_groups=[[0, 1, 2, 3, 4, 5, 6, 7]],
            ins=[em_loc[:]], outs=[em_red[:]])

        # ======== Phase E: gather this core's CRF block, transpose ====
        TOKC = S * BCRF
        pp2 = es.enter_context(tc.tile_pool(name="pp2", bufs=1))
        em_sb = pp2.tile([TT, TOKC], F32, tag="em_sb", name="em_sb")
        trans_sb = pp2.tile([TT, TT], F32, tag="trans_sb")
        nc.sync.dma_start(
            trans_sb[:],
            blob_f32[BF_TRANS:BF_TRANS + TT * TT, :].rearrange(
                "(a b) 1 -> a b", a=TT))
        with tc.tile_pool(name="gat2", bufs=4) as g2, \
             tc.tile_pool(name="pgat2", bufs=4, space="PSUM") as pg2:
            for i in range(TOKC // 128):
                # core c's CRF block is em rows [c*4096, (c+1)*4096) — one
                # contiguous dynamic-offset DMA per tile, (b, t) order
                gblk = g2.tile([128, TT], BF16, tag="gblk")
                nc.sync.dma_start(
                    gblk[:],
                    em_red[bass.ds(pid * (BCRF * S) + i * 128, 128), :])
                tpb = pg2.tile([TT, 128], BF16, tag="tpb")
                nc.tensor.transpose(out=tpb[:], in_=gblk[:],
                                    identity=identb[:])
                nc.vector.tensor_copy(em_sb[:, i * 128:(i + 1) * 128], tpb[:])

        # ======== Phase F: gold score ====
        NCH2 = TOKC // 512
        ones_col = cpool.tile([TT, 1], F32)
        nc.vector.memset(ones_col[:], 1.0)
        ones_rf = cpool.tile([1, 128], F32)
        nc.vector.memset(ones_rf[:], 1.0)
        iota_i = cpool.tile([TT, 1], I32)
        nc.gpsimd.iota(iota_i[:], pattern=[[0, 1]], base=0, channel_multiplier=1)
        iota_f = cpool.tile([TT, 1], F32)
        nc.vector.tensor_copy(iota_f[:], iota_i[:])
        ppg = es.enter_context(tc.tile_pool(name="ppg", bufs=1))
        gold_emit_tok = ppg.tile([1, TOKC], F32, tag="g_emit")
        gold_trans_tok = ppg.tile([1, TOKC], F32, tag="g_trans")
        with tc.tile_pool(name="gold", bufs=3) as gld, \
             tc.tile_pool(name="pgold", bufs=1, space="PSUM") as pgd:
            for ch in range(NCH2):
                sl = slice(ch * 512, (ch + 1) * 512)
                masks = {}
                for nm, tbase in (("next", BF_TNEXT), ("prev", BF_TPREV)):
                    trow = gld.tile([1, 512], F32, tag=f"trow{nm}")
                    nc.sync.dma_start(
                        trow[:],
                        blob_f32[tbase + sl.start:tbase + sl.stop, :]
                        .rearrange("n 1 -> 1 n"))
                    tbc = pgd.tile([TT, 512], F32, tag=f"tbc{nm}", name=f"tbc{nm}{ch}")
                    nc.tensor.matmul(tbc[:], ones_rf[:1, :TT], trow[:], start=True, stop=True)
                    mask = gld.tile([TT, 512], F32, tag=f"mask{nm}")
                    nc.vector.tensor_tensor(
                        out=mask[:], in0=tbc[:],
                        in1=iota_f[:, :1].to_broadcast([TT, 512]),
                        op=ALU.is_equal)
                    masks[nm] = mask
                me = gld.tile([TT, 512], F32, tag="me")
                nc.vector.tensor_mul(me[:], masks["next"][:], em_sb[:, sl])
                srow = pgd.tile([1, 512], F32, tag="srow", name=f"srow{ch}")
                nc.tensor.matmul(srow[:], ones_col[:, :1], me[:], start=True, stop=True)
                nc.vector.tensor_copy(gold_emit_tok[:, sl], srow[:])
                ups = pgd.tile([TT, 512], F32, tag="ups", name=f"ups{ch}")
                nc.tensor.matmul(ups[:], trans_sb[:], masks["prev"][:], start=True, stop=True)
                mu = gld.tile([TT, 512], F32, tag="mu")
                nc.vector.tensor_mul(mu[:], masks["next"][:], ups[:])
                srow2 = pgd.tile([1, 512], F32, tag="srow2", name=f"srow2{ch}")
                nc.tensor.matmul(srow2[:], ones_col[:, :1], mu[:], start=True, stop=True)
                nc.vector.tensor_copy(gold_trans_tok[:, sl], srow2[:])

        # ======== Phase G: CRF forward in exp space ====
        with tc.tile_pool(name="crf", bufs=2) as crf, \
             tc.tile_pool(name="crfst", bufs=1) as crfst, \
             tc.tile_pool(name="pcrf", bufs=6, space="PSUM") as pcf:

            def psc(name):
                return pcf.tile([TT, 4], F32, tag="ps", name=name)

            e_mat = crfst.tile([TT, TT], F32, tag="e_mat")
            negc = crfst.tile([TT, 1], F32, tag="negc")
            nc.vector.memset(negc[:], -C_TRANS)
            nc.scalar.activation(e_mat[:], trans_sb[:], AF.Exp, bias=negc[:, :1])
            for ch in range(NCH2):
                nc.scalar.activation(em_sb[:, ch * 512:(ch + 1) * 512],
                                     em_sb[:, ch * 512:(ch + 1) * 512], AF.Exp)
            shift_col = crf.tile([TT, 1], F32, tag="shift_col")
            nc.vector.memset(shift_col[:], 0.0)
            nc.vector.memset(shift_col[:1, :], -NEG)
            ew = crf.tile([TT, TT], F32, tag="ew")
            nc.scalar.activation(ew[:], trans_sb[:], AF.Exp, bias=shift_col[:, :1])
            wps = psc("wps")
            nc.tensor.matmul(wps[:, :1], ew[:], ones_col[:, :1], start=True, stop=True)
            w_col = crf.tile([TT, 1], F32, tag="w_col")
            nc.vector.tensor_copy(w_col[:], wps[:, :1])
            NCHAIN, CW = 2, BCRF // 2
            a_ch, lz_ch = [], []
            for i2 in range(NCHAIN):
                a_i = crfst.tile([TT, CW], F32, tag=f"a{i2}", name=f"a{i2}")
                nc.vector.tensor_scalar_mul(
                    a_i[:], em_sb[:, i2 * CW * S:(i2 + 1) * CW * S:S],
                    w_col[:, :1])
                lz_i = crfst.tile([1, CW], F32, tag=f"lz{i2}", name=f"lz{i2}")
                nc.vector.memset(lz_i[:], NEG)
                a_ch.append(a_i)
                lz_ch.append(lz_i)

            def renorm(i2, tag):
                a_i, lz_i = a_ch[i2], lz_ch[i2]
                sps = psc(f"sps{i2}_{tag}")
                nc.tensor.matmul(sps[:1, :CW], ones_col[:, :1], a_i[:], start=True, stop=True)
                lns = crf.tile([1, CW], F32, tag=f"lns{i2}", name=f"lns{i2}_{tag}")
                nc.scalar.activation(lns[:], sps[:1, :CW], AF.Ln)
                nc.vector.tensor_add(lz_i[:], lz_i[:], lns[:])
                rs_ = crf.tile([1, CW], F32, tag=f"rs{i2}", name=f"rs{i2}_{tag}")
                nc.vector.reciprocal(rs_[:], sps[:1, :CW])
                rbc = psc(f"rbc{i2}_{tag}")
                nc.tensor.matmul(rbc[:, :CW], ones_rf[:1, :TT], rs_[:], start=True, stop=True)
                nc.vector.tensor_mul(a_i[:], a_i[:], rbc[:, :CW])

            for t in range(1, S):
                for i2 in range(NCHAIN):
                    aps = psc(f"aps{i2}_{t}")
                    nc.tensor.matmul(aps[:, :CW], e_mat[:], a_ch[i2][:], start=True, stop=True)
                    nc.vector.tensor_mul(
                        a_ch[i2][:], aps[:, :CW],
                        em_sb[:, i2 * CW * S + t:(i2 + 1) * CW * S:S])
                    if (t + 3 * i2) % RENORM == 0:
                        renorm(i2, t)

            logz = crf.tile([1, BCRF], F32, tag="logz")
            for i2 in range(NCHAIN):
                sfin = psc(f"sfin{i2}")
                nc.tensor.matmul(sfin[:1, :CW], ones_col[:, :1], a_ch[i2][:], start=True, stop=True)
                lnf = crf.tile([1, CW], F32, tag=f"lnf{i2}", name=f"lnf{i2}")
                nc.scalar.activation(lnf[:], sfin[:1, :CW], AF.Ln)
                nc.vector.tensor_add(logz[:, i2 * CW:(i2 + 1) * CW], lz_ch[i2][:], lnf[:])
            nc.vector.tensor_scalar_add(logz[:], logz[:], float(S - 1) * C_TRANS)
            gsum = crf.tile([1, BCRF], F32, tag="gsum")
            nc.vector.tensor_reduce(
                out=gsum[:],
                in_=gold_emit_tok[:1].rearrange("p (b t) -> p b t", b=BCRF),
                axis=mybir.AxisListType.X, op=ALU.add)
            gsum2 = crf.tile([1, BCRF], F32, tag="gsum2")
            nc.vector.tensor_reduce(
                out=gsum2[:],
                in_=gold_trans_tok[:1].rearrange("p (b t) -> p b t", b=BCRF),
                axis=mybir.AxisListType.X, op=ALU.add)
            nc.vector.tensor_add(gsum[:], gsum[:], gsum2[:])
            res = crf.tile([1, BCRF], F32, tag="res")
            nc.vector.tensor_sub(res[:], logz[:], gsum[:])
            nc.sync.dma_start(out_dram[:], res[:])

    nc.compile()
    return nc


def build_crf_program(nsteps=S):
    """Launch-2 program: gold score + CRF forward for 8 batches (em given)."""
    Sq = nsteps
    TOKq = Sq * BCRF
    NCH = max(TOKq // 512, 1)
    CHW = min(TOKq, 512)
    nc = bacc.Bacc("TRN2", target_bir_lowering=False, debug=False,
                   num_devices=NCORES)

    din = lambda name, shp, dt=F32: nc.dram_tensor(name, shp, dt, kind="ExternalInput").ap()
    em_in = din("em_in", [TT, TOKq])
    tags_next = din("tags_next", [TOKq, 1])
    tags_prev = din("tags_prev", [TOKq, 1])
    transT = din("transT", [TT, TT])
    out_dram = nc.dram_tensor("out", [1, BCRF], F32, kind="ExternalOutput").ap()

    with tile.TileContext(nc) as tc, ExitStack() as es:
        cpool = es.enter_context(tc.tile_pool(name="const", bufs=1))
        ones_col = cpool.tile([TT, 1], F32)
        nc.vector.memset(ones_col[:], 1.0)
        ones_row = cpool.tile([1, 128], F32)
        nc.vector.memset(ones_row[:], 1.0)
        iota_i = cpool.tile([TT, 1], I32)
        nc.gpsimd.iota(iota_i[:], pattern=[[0, 1]], base=0, channel_multiplier=1)
        iota_f = cpool.tile([TT, 1], F32)
        nc.vector.tensor_copy(iota_f[:], iota_i[:])

        pp = es.enter_context(tc.tile_pool(name="persist", bufs=1))
        trans_sb = pp.tile([TT, TT], F32, tag="trans_sb")
        nc.sync.dma_start(trans_sb[:], transT[:])
        em_sb = pp.tile([TT, TOKq], F32, tag="em_sb")
        nc.sync.dma_start(em_sb[:], em_in[:])

        # ---- gold score ----
        ppg = es.enter_context(tc.tile_pool(name="ppg", bufs=1))
        gold_emit_tok = ppg.tile([1, TOKq], F32, tag="g_emit")
        gold_trans_tok = ppg.tile([1, TOKq], F32, tag="g_trans")
        with tc.tile_pool(name="gold", bufs=3) as gld, \
             tc.tile_pool(name="pgold", bufs=1, space="PSUM") as pgd:
            for ch in range(NCH):
                sl = slice(ch * CHW, (ch + 1) * CHW)
                masks = {}
                for nm, tarr in (("next", tags_next), ("prev", tags_prev)):
                    trow = gld.tile([1, CHW], F32, tag=f"trow{nm}")
                    nc.sync.dma_start(trow[:], tarr[sl, :].rearrange("n 1 -> 1 n"))
                    tbc = pgd.tile([TT, CHW], F32, tag=f"tbc{nm}", name=f"tbc{nm}{ch}")
                    nc.tensor.matmul(tbc[:], ones_row[:1, :TT], trow[:], start=True, stop=True)
                    mask = gld.tile([TT, CHW], F32, tag=f"mask{nm}")
                    nc.vector.tensor_tensor(
                        out=mask[:], in0=tbc[:],
                        in1=iota_f[:, :1].to_broadcast([TT, CHW]),
                        op=ALU.is_equal)
                    masks[nm] = mask
                me = gld.tile([TT, CHW], F32, tag="me")
                nc.vector.tensor_mul(me[:], masks["next"][:], em_sb[:, sl])
                srow = pgd.tile([1, CHW], F32, tag="srow", name=f"srow{ch}")
                nc.tensor.matmul(srow[:], ones_col[:, :1], me[:], start=True, stop=True)
                nc.vector.tensor_copy(gold_emit_tok[:, sl], srow[:])
                ups = pgd.tile([TT, CHW], F32, tag="ups", name=f"ups{ch}")
                nc.tensor.matmul(ups[:], trans_sb[:], masks["prev"][:], start=True, stop=True)
                mu = gld.tile([TT, CHW], F32, tag="mu")
                nc.vector.tensor_mul(mu[:], masks["next"][:], ups[:])
                srow2 = pgd.tile([1, CHW], F32, tag="srow2", name=f"srow2{ch}")
                nc.tensor.matmul(srow2[:], ones_col[:, :1], mu[:], start=True, stop=True)
                nc.vector.tensor_copy(gold_trans_tok[:, sl], srow2[:])

        # ---- CRF forward in exp space ----
        with tc.tile_pool(name="crf", bufs=2) as crf, \
             tc.tile_pool(name="crfst", bufs=1) as crfst, \
             tc.tile_pool(name="pcrf", bufs=6, space="PSUM") as pcf:

            def psc(name):
                return pcf.tile([TT, 4], F32, tag="ps", name=name)
            e_mat = crfst.tile([TT, TT], F32, tag="e_mat")
            negc = crfst.tile([TT, 1], F32, tag="negc")
            nc.vector.memset(negc[:], -C_TRANS)
            nc.scalar.activation(e_mat[:], trans_sb[:], AF.Exp, bias=negc[:, :1])
            for ch in range(NCH):
                nc.scalar.activation(em_sb[:, ch * CHW:(ch + 1) * CHW],
                                     em_sb[:, ch * CHW:(ch + 1) * CHW], AF.Exp)
            shift_col = crf.tile([TT, 1], F32, tag="shift_col")
            nc.vector.memset(shift_col[:], 0.0)
            nc.vector.memset(shift_col[:1, :], -NEG)
            ew = crf.tile([TT, TT], F32, tag="ew")
            nc.scalar.activation(ew[:], trans_sb[:], AF.Exp, bias=shift_col[:, :1])
            wps = psc("wps")
            nc.tensor.matmul(wps[:, :1], ew[:], ones_col[:, :1], start=True, stop=True)
            w_col = crf.tile([TT, 1], F32, tag="w_col")
            nc.vector.tensor_copy(w_col[:], wps[:, :1])
            # 4 independent alpha chains of 2 batches each; interleaving hides
            # the PE<->DVE round-trip latency of the sequential scan
            NCHAIN, CW = 2, BCRF // 2
            a_ch, lz_ch = [], []
            for i in range(NCHAIN):
                a_i = crfst.tile([TT, CW], F32, tag=f"a{i}", name=f"a{i}")
                nc.vector.tensor_scalar_mul(
                    a_i[:], em_sb[:, i * CW:(i + 1) * CW], w_col[:, :1])
                lz_i = crfst.tile([1, CW], F32, tag=f"lz{i}", name=f"lz{i}")
                nc.vector.memset(lz_i[:], NEG)
                a_ch.append(a_i)
                lz_ch.append(lz_i)

            def renorm(i, tag):
                a_i, lz_i = a_ch[i], lz_ch[i]
                sps = psc(f"sps{i}_{tag}")
                nc.tensor.matmul(sps[:1, :CW], ones_col[:, :1], a_i[:], start=True, stop=True)
                lns = crf.tile([1, CW], F32, tag=f"lns{i}", name=f"lns{i}_{tag}")
                nc.scalar.activation(lns[:], sps[:1, :CW], AF.Ln)
                nc.vector.tensor_add(lz_i[:], lz_i[:], lns[:])
                rs_ = crf.tile([1, CW], F32, tag=f"rs{i}", name=f"rs{i}_{tag}")
                nc.vector.reciprocal(rs_[:], sps[:1, :CW])
                rbc = psc(f"rbc{i}_{tag}")
                nc.tensor.matmul(rbc[:, :CW], ones_row[:1, :TT], rs_[:], start=True, stop=True)
                nc.vector.tensor_mul(a_i[:], a_i[:], rbc[:, :CW])

            for t in range(1, Sq):
                for i in range(NCHAIN):
                    aps = psc(f"aps{i}_{t}")
                    nc.tensor.matmul(aps[:, :CW], e_mat[:], a_ch[i][:], start=True, stop=True)
                    nc.vector.tensor_mul(
                        a_ch[i][:], aps[:, :CW],
                        em_sb[:, t * BCRF + i * CW:t * BCRF + (i + 1) * CW])
                    if (t + 3 * i) % RENORM == 0:
                        renorm(i, t)

            logz = crf.tile([1, BCRF], F32, tag="logz")
            for i in range(NCHAIN):
                sfin = psc(f"sfin{i}")
                nc.tensor.matmul(sfin[:1, :CW], ones_col[:, :1], a_ch[i][:], start=True, stop=True)
                lnf = crf.tile([1, CW], F32, tag=f"lnf{i}", name=f"lnf{i}")
                nc.scalar.activation(lnf[:], sfin[:1, :CW], AF.Ln)
                nc.vector.tensor_add(logz[:, i * CW:(i + 1) * CW], lz_ch[i][:], lnf[:])
            nc.vector.tensor_scalar_add(logz[:], logz[:], float(Sq - 1) * C_TRANS)
            gsum = crf.tile([1, BCRF], F32, tag="gsum")
            nc.vector.tensor_reduce(
                out=gsum[:],
                in_=gold_emit_tok[:1].rearrange("p (b t) -> p b t", b=BCRF),
                axis=mybir.AxisListType.X, op=ALU.add)
            gsum2 = crf.tile([1, BCRF], F32, tag="gsum2")
            nc.vector.tensor_reduce(
                out=gsum2[:],
                in_=gold_trans_tok[:1].rearrange("p (b t) -> p b t", b=BCRF),
                axis=mybir.AxisListType.X, op=ALU.add)
            nc.vector.tensor_add(gsum[:], gsum[:], gsum2[:])
            res = crf.tile([1, BCRF], F32, tag="res")
            nc.vector.tensor_sub(res[:], logz[:], gsum[:])
            nc.sync.dma_start(out_dram[:], res[:])

    nc.compile()
    return nc


_CACHE = {}


def _get_program(nsteps=S):
    key = ("lstm", nsteps)
    if key not in _CACHE:
        _CACHE[key] = build_lstm_program(nsteps)
    return _CACHE[key]


def _get_program_crf(nsteps=S):
    key = ("crf", nsteps)
    if key not in _CACHE:
        _CACHE[key] = build_crf_program(nsteps)
    return _CACHE[key]


GATE_PERM = [1, 0, 3, 2]  # (i, f, g, o) -> (f, i, o, g), chunks of H


def _permute_gates(wT):
    # wT: [in_dim, 4H] with pytorch gate order i,f,g,o -> i,f,o,g
    chunks = [wT[:, k * H:(k + 1) * H] for k in range(4)]
    return np.concatenate([chunks[p] for p in GATE_PERM], axis=1)


def make_in_maps_lstm(sentences, tags, emb, w_ih_f, w_hh_f, b_ih_f, b_hh_f,
                      w_ih_b, w_hh_b, b_ih_b, b_hh_b, w_out, b_out,
                      transitions, nsteps=S):
    f32 = np.float32
    np8 = mybir.dt.np(FP8)
    sentences = np.asarray(sentences)
    tags = np.asarray(tags)
    tags_ext = np.concatenate([np.zeros((B, 1), tags.dtype), tags], axis=1)
    # compact the vocab to the rows this batch actually uses (<= B*S = VPAD)
    uniq, inv = np.unique(sentences, return_inverse=True)
    sentences = inv.reshape(sentences.shape)
    emb_pad = np.zeros((VPAD, E), np8)
    emb_pad[:len(uniq)] = np.asarray(emb, f32)[uniq].astype(np8)
    w_out = np.asarray(w_out, f32)
    woutT = {"f": np.ascontiguousarray(w_out[:, :H].T).astype(np8),
             "b": np.ascontiguousarray(w_out[:, H:].T).astype(np8)}
    pack = np.zeros((WG_TOT, 512), np8)
    shared = {}
    for di, (d, w_ih, w_hh, b_ih, b_hh) in enumerate((
            ("f", w_ih_f, w_hh_f, b_ih_f, b_hh_f),
            ("b", w_ih_b, w_hh_b, b_ih_b, b_hh_b))):
        wih8 = np.ascontiguousarray(
            _permute_gates(np.asarray(w_ih, f32).T) * WSCALE).astype(np8)
        whh8 = np.ascontiguousarray(
            _permute_gates(np.asarray(w_hh, f32).T) * WSCALE).astype(np8)
        db = di * WG_DIR
        pack[db + WG_WIH:db + WG_WIH + 2048] = wih8.reshape(2048, 512)
        pack[db + WG_WHH:db + WG_WHH + 2048] = whh8.reshape(2048, 512)
        pack[db + WG_WOUT:db + WG_WOUT + 64] = woutT[d].reshape(64, 512)
        shared[d] = dict(
            bias=np.ascontiguousarray(_permute_gates(
                (np.asarray(b_ih, f32) + np.asarray(b_hh, f32)).reshape(1, G4))
            ).reshape(-1) * WSCALE,
        )
    bout = {"f": np.asarray(b_out, f32).reshape(-1),
            "b": np.zeros(TT, f32)}
    transT_flat = np.ascontiguousarray(
        np.asarray(transitions, f32).T).reshape(-1)
    in_maps = []
    ts_can = np.arange(nsteps)
    for c in range(NCORES):
        d = "f" if c < 4 else "b"
        q = c % 4
        sl = sentences[q * BC:(q + 1) * BC, :nsteps]     # [16, Sq]
        tcan = ts_can if d == "f" else ts_can[::-1]
        if d == "b":
            sl = sl[:, ::-1]
        # local token j = t_local*16 + b -> global em row (16q+b)*S + t_canonical
        g = (16 * q + np.arange(BC))[None, :] * S + tcan[:, None]   # [Sq, 16]
        # CRF block for this core: batches [8c, 8c+8), tags in (b, t) order
        # to match the contiguous dynamic-offset em read
        g0 = c * BCRF
        bsl = slice(g0, g0 + BCRF)
        blob_i = np.concatenate([
            sl.T.reshape(-1), g.reshape(-1)]).astype(np.int32)
        blob_f = np.concatenate([
            shared[d]["bias"], bout[d], transT_flat,
            tags[bsl, :].reshape(-1).astype(f32),
            tags_ext[bsl, :S].reshape(-1).astype(f32)]).astype(f32)
        in_maps.append(dict(
            blob_i32=np.ascontiguousarray(blob_i.reshape(-1, 1)),
            emb_shard=np.ascontiguousarray(emb_pad[c * VS:(c + 1) * VS]),
            wg_shard=np.ascontiguousarray(
                pack[c * WG_PER:(c + 1) * WG_PER]),
            blob_f32=np.ascontiguousarray(blob_f.reshape(-1, 1)),
        ))
    return in_maps


def make_in_maps_crf(em_quads, tags, transitions, nsteps=S):
    """em_quads: list of 4 arrays [TT, Sq*16] canonical (t,b) order."""
    f32 = np.float32
    tags = np.asarray(tags)
    tags_ext = np.concatenate([np.zeros((B, 1), tags.dtype), tags], axis=1)
    transT = np.ascontiguousarray(np.asarray(transitions, f32).T)
    in_maps = []
    for c in range(NCORES):
        g0 = c * BCRF                       # global batch start
        q, b0 = g0 // BC, g0 % BC
        em_q = em_quads[q].reshape(TT, nsteps, BC)
        em_lin = np.ascontiguousarray(em_q[:, :, b0:b0 + BCRF].reshape(TT, -1))
        bsl = slice(g0, g0 + BCRF)
        in_maps.append(dict(
            em_in=em_lin,
            tags_next=np.ascontiguousarray(
                tags[bsl, :nsteps].T.reshape(-1, 1).astype(f32)),
            tags_prev=np.ascontiguousarray(
                tags_ext[bsl, :nsteps].T.reshape(-1, 1).astype(f32)),
            transT=transT,
        ))
    return in_maps


def kernel(sentences, tags, lengths, emb, w_ih_f, w_hh_f, b_ih_f, b_hh_f,
           w_ih_b, w_hh_b, b_ih_b, b_hh_b, w_out, b_out, transitions):
    nc1 = _get_program(S)
    maps1 = make_in_maps_lstm(sentences, tags, emb,
                              w_ih_f, w_hh_f, b_ih_f, b_hh_f,
                              w_ih_b, w_hh_b, b_ih_b, b_hh_b, w_out, b_out,
                              transitions)
    res = run_bass_kernel_spmd(nc1, maps1, core_ids=list(range(NCORES)))
    parts = np.concatenate([r["out"].reshape(-1) for r in res.results])
    return np.float32(parts.mean())


def measure_program_exec_ns(nc, in_maps, repeats=25):
    """Device-resident repeat timing of a compiled program across its cores.

    Mirrors bass2jax.run_bass_via_pjrt's multi-core path but keeps inputs on
    device so per-call wall ~= dispatch + execution. Returns 25th-pct ns.
    """
    import time
    import jax
    from jax.sharding import Mesh, PartitionSpec
    from jax.experimental.shard_map import shard_map
    from concourse import bass2jax, mybir as _mb

    bass2jax.install_neuronx_cc_hook()
    partition_name = (nc.partition_id_tensor.name
                      if nc.partition_id_tensor else None)
    in_names, out_names, out_avals, zero_outs = [], [], [], []
    for alloc in nc.m.functions[0].allocations:
        if not isinstance(alloc, _mb.MemoryLocationSet):
            continue
        name = alloc.memorylocations[0].name
        if alloc.kind == "ExternalInput":
            if name != partition_name:
                in_names.append(name)
        elif alloc.kind == "ExternalOutput":
            out_names.append(name)
            shape = tuple(alloc.tensor_shape)
            dtype = _mb.dt.np(alloc.dtype)
            out_avals.append(jax.core.ShapedArray(shape, dtype))
            zero_outs.append(np.zeros(shape, dtype))
    n_params = len(in_names)
    all_names = in_names + out_names
    if partition_name is not None:
        all_names = all_names + [partition_name]

    def _body(*args):
        operands = list(args)
        if partition_name is not None:
            operands.append(bass2jax.partition_id_tensor())
        outs = bass2jax._bass_exec_p.bind(
            *operands, out_avals=tuple(out_avals), in_names=tuple(all_names),
            out_names=tuple(out_names), lowering_input_output_aliases=(),
            sim_require_finite=True, sim_require_nnan=True, nc=nc)
        return tuple(outs)

    ncores = len(in_maps)
    devices = jax.devices()[:ncores]
    mesh = Mesh(np.asarray(devices), ("core",))
    n_outs = len(out_names)
    sharded = jax.jit(
        shard_map(_body, mesh=mesh,
                  in_specs=(PartitionSpec("core"),) * (n_params + n_outs),
                  out_specs=(PartitionSpec("core"),) * n_outs,
                  check_rep=False),
        keep_unused=True)
    concat_in = [
        np.concatenate([np.asarray(in_maps[c][nm]) for c in range(ncores)], axis=0)
        for nm in in_names]
    concat_zeros = [np.zeros((ncores * z.shape[0], *z.shape[1:]), z.dtype)
                    for z in zero_outs]
    dev_in = [jax.device_put(a) for a in concat_in]
    dev_zero = [jax.device_put(z) for z in concat_zeros]
    r = sharded(*dev_in, *dev_zero)
    jax.block_until_ready(r)
    samples = []
    for _ in range(repeats):
        t0 = time.perf_counter()
        r = sharded(*dev_in, *dev_zero)
        jax.block_until_ready(r)
        samples.append((time.perf_counter() - t0) * 1e9)
    samples.sort()
    return samples[len(samples) // 4]

